# revision 14
# baseline (speedup 1.0000x reference)
"""MoE routed expert matmul on 8 Trainium2 NeuronCores.

Problem: out[n] = input[n] @ w[inds[n]] + b[inds[n]]
  input [262144, 32] f32, inds [262144] i32 (1024 experts), w [1024, 32, 32], b [1024, 1, 32]

Strategy (K-stacked expert quads; host does routing/layout only — all FLOPs
on device):
  * Host sorts the 1024 experts by global token count (ascending) and chunks
    them into 32 quad-groups of 32 experts with near-equal counts.  Chunk q
    supplies one expert to each (core, band) pair: expert chunks[q][4k + r]
    goes to core k, quad q, band r (r in 0..3).  Every core runs the same
    program over its own 32 quads; quad q's column width Q[q] = max token
    count in the chunk (global max, so the SPMD shapes match), rounded up to
    8.  Count-matched chunks keep padding to a few percent.
  * Activation layout xt [128, TOTW] fp16: token t of (quad q, band r) sits
    at column X[q] + t, rows 32r..32r+32 (its 32 features).  Each column
    carries up to 4 tokens (one per band) — full 128-row density.
  * Weights upload as block-diagonal K=64 stacks (wq, 0.5 MB): for each quad
    and half h, a [64, 64] tile holds experts (q, 2h) and (q, 2h+1) on the
    diagonal.  Two [K=64, M=64, N=Q] matmuls per quad (tile_position (0,0) /
    (64,64)) then compute all 4 bands' tokens — each activation column
    streams through the PE twice instead of 4x (vs per-expert 32x32 tiles),
    and the off-diagonal zeros kill the cross-expert terms.
  * The PSUM result + per-quad bias column goes to an fp16 output tile
    (Scalar/Vector alternating), stored to DRAM in per-4-quad groups on
    alternating DMA rings (GpSimd SWDGE / Scalar HWDGE).  fp16 I/O halves
    DMA traffic vs f32; per-core HBM bytes ~4.8 MB -> ~13.3 us at 360 B/ns.
  * Host scatters the sorted outputs back to original token order.

Layouts (core k, quad q, band r = 2h + s, expert e = chunks[q][4k + r]):
  xt [128, TOTW]  xt[32r+i, X[q] + t]        = x[token t of e, feat i]  (fp16)
  wq [128, 2048]  wq[64h+32s+i, 64q+32s+o]   = w[e, i, o], 0 off-diag   (fp16)
  bp [128, 32]    bp[32r+o, q]               = b[e, 0, o]               (f32)
  ot [128, TOTW]  ot[32r+o, X[q] + t]        = out[token t of e, feat o](fp16)
"""

import numpy as np

import concourse.bass as bass
import concourse.mybir as mybir
import concourse.tile as tile
from concourse import bacc
from concourse.bass_utils import run_bass_kernel_spmd

N_TOK = 262144
E = 1024
F = 32
O = 32
NCORES = 8
NQUAD = 32  # quads per core; 4 experts each = 128 experts/core
GQ = 4  # quads per load/store group
NG = NQUAD // GQ
F32 = mybir.dt.float32
MM_DT = mybir.dt.float16
OT_DT = mybir.dt.float16

N_WARM = 8  # PE ramp warm-up matmuls
WARM_N = 160  # free-dim length of each warm-up matmul

_programs: dict[tuple, "bacc.Bacc"] = {}


class _CapacityOverflow(Exception):
    """A single expert got >512 tokens (~16 sigma out for uniform routing at
    256 tokens/expert).  Handled by a host fallback so kernel() still
    returns a correct result."""


def _plan(counts):
    """Chunk experts into count-matched quads; per-quad widths and offsets."""
    order_e = np.argsort(counts, kind="stable")  # ascending counts
    # chunk q holds 32 count-matched experts; descending so the pipeline
    # tail (last-stored groups) drains on the smallest transfers
    chunks = order_e.reshape(NQUAD, 32)[::-1]
    Q = np.maximum(16, ((counts[chunks[:, -1]] + 7) // 8) * 8)  # [NQUAD]
    # uniform width within each 4-quad group so one batched bias op (and one
    # contiguous PSUM slot stride) covers the group; count-sorted chunks make
    # the extra padding ~1%
    Q = np.repeat(Q.reshape(NG, GQ).max(axis=1), GQ)
    if Q.max() > 512:
        raise _CapacityOverflow(int(counts.max()))
    X = np.zeros(NQUAD + 1, dtype=np.int64)
    np.cumsum(Q, out=X[1:])
    TOTW = int(X[-1])
    j = np.arange(32)
    e_quad = np.empty(E, dtype=np.int64)
    e_core = np.empty(E, dtype=np.int64)
    e_band = np.empty(E, dtype=np.int64)
    e_quad[chunks] = np.arange(NQUAD)[:, None]
    e_core[chunks] = (j // 4)[None, :]
    e_band[chunks] = (j % 4)[None, :]
    return Q.astype(np.int64), X, TOTW, e_quad, e_core, e_band


def _build(Q, X, TOTW) -> "bacc.Bacc":
    nc = bacc.Bacc("TRN2", target_bir_lowering=False, debug=False, num_devices=NCORES)
    xt = nc.declare_dram_parameter("xt", [128, TOTW], MM_DT, isOutput=False)
    wq = nc.declare_dram_parameter("wq", [128, NQUAD * 64], MM_DT, isOutput=False)
    bp = nc.declare_dram_parameter("bp", [128, NQUAD], F32, isOutput=False)
    ot = nc.declare_dram_parameter("ot", [128, TOTW], OT_DT, isOutput=True)

    with tile.TileContext(nc) as tc:
        with (
            tc.tile_pool(name="const", bufs=1) as c_pool,
            tc.tile_pool(name="xt", bufs=4) as xt_pool,
            tc.tile_pool(name="out", bufs=4) as out_pool,
            tc.tile_pool(name="psm", bufs=2, space="PSUM") as psm_pool,
        ):
            wq_t = c_pool.tile([128, NQUAD * 64], MM_DT)
            bp_t = c_pool.tile([128, NQUAD], F32)
            warm_t = c_pool.tile([128, WARM_N], MM_DT)

            # loads: sync (SP HWDGE) carries the first wq half + all xt;
            # gpsimd (SWDGE) the second wq half and the bias
            wqh = NQUAD * 64 // 2
            nc.sync.dma_start(out=wq_t[:, :wqh], in_=wq[:, :wqh])
            nc.gpsimd.dma_start(out=wq_t[:, wqh:], in_=wq[:, wqh:])
            nc.gpsimd.dma_start(out=bp_t[:], in_=bp[:])

            # PE ramp warm-up on a memset scratch tile (PSUM never read)
            nc.vector.memset(warm_t[:], 0.0)
            warm_ps = psm_pool.tile(
                [128, WARM_N], F32, space="PSUM", name="warm_ps", tag="psm"
            )
            for _ in range(N_WARM):
                nc.tensor.matmul(
                    out=warm_ps[0:32, :],
                    lhsT=warm_t[0:32, 0:32],
                    rhs=warm_t[0:32, :],
                    start=True,
                    stop=True,
                    tile_position=(0, 0),
                )

            xt_tiles = {}
            o_tiles = {}

            def load_group(g, split=1):
                a, bnd = int(X[GQ * g]), int(X[GQ * (g + 1)])
                t = xt_pool.tile([128, bnd - a], MM_DT, name="xt_t", tag="xt_t")
                w_ = bnd - a
                for s in range(split):
                    c0, c1 = s * w_ // split, (s + 1) * w_ // split
                    nc.sync.dma_start(out=t[:, c0:c1], in_=xt[:, a + c0 : a + c1])
                xt_tiles[g] = t

            load_group(0, split=2)
            load_group(1)

            for g in range(NG):
                if g + 2 < NG:
                    load_group(g + 2)
                a, bnd = int(X[GQ * g]), int(X[GQ * (g + 1)])
                Qg = int(Q[GQ * g])
                o_t = out_pool.tile([128, bnd - a], OT_DT, name="o_t", tag="o_t")
                # one PSUM tile per group; each quad in its own 512-col bank
                psm = psm_pool.tile(
                    [128, GQ * 512], F32, space="PSUM", name="psm", tag="psm"
                )
                for qi in range(GQ):
                    q = GQ * g + qi
                    for h in range(2):
                        nc.tensor.matmul(
                            out=psm[64 * h : 64 * h + 64, 512 * qi : 512 * qi + Qg],
                            lhsT=wq_t[64 * h : 64 * h + 64, 64 * q : 64 * q + 64],
                            rhs=xt_tiles[g][
                                64 * h : 64 * h + 64, qi * Qg : (qi + 1) * Qg
                            ],
                            start=True,
                            stop=True,
                            tile_position=(64 * h, 64 * h),
                        )
                # batched bias + fp16 down-convert: one op per group on DVE
                # (tensor_tensor w/ broadcast bias); ACT takes a couple of
                # groups as 4 per-quad activation ops to share the load
                if g % 4 != 3:
                    psm_view = psm[:, :].rearrange("p (c t) -> p c t", c=GQ)[
                        :, :, :Qg
                    ]
                    bias_view = bp_t[:, GQ * g : GQ * (g + 1), None].to_broadcast(
                        [128, GQ, Qg]
                    )
                    out_view = o_t[:, :].rearrange("p (c t) -> p c t", c=GQ)
                    nc.vector.tensor_tensor(
                        out=out_view,
                        in0=psm_view,
                        in1=bias_view,
                        op=mybir.AluOpType.add,
                    )
                else:
                    for qi in range(GQ):
                        q = GQ * g + qi
                        nc.scalar.activation(
                            o_t[:, qi * Qg : (qi + 1) * Qg],
                            psm[:, 512 * qi : 512 * qi + Qg],
                            mybir.ActivationFunctionType.Identity,
                            bias=bp_t[:, q : q + 1],
                            scale=1.0,
                        )
                ring = nc.gpsimd if g % 2 == 0 else nc.scalar
                ring.dma_start(out=ot[:, a:bnd], in_=o_t[:])

    nc.compile()
    return nc


def _pack(x, inds, w, b):
    """Host-side routing: sort tokens by expert, build per-core device arrays."""
    counts = np.bincount(inds, minlength=E)
    Q, X, TOTW, e_quad, e_core, e_band = _plan(counts)

    order = np.argsort(inds, kind="stable")
    sorted_inds = inds[order]
    starts = np.zeros(E, dtype=np.int64)
    np.cumsum(counts[:-1], out=starts[1:])
    slot = np.arange(N_TOK, dtype=np.int64) - starts[sorted_inds]

    k_tok = e_core[sorted_inds]
    r_tok = e_band[sorted_inds]
    col_tok = X[e_quad[sorted_inds]] + slot

    mdt = mybir.dt.np(MM_DT)
    xt_all = np.zeros((NCORES, 4, F, TOTW), dtype=mdt)
    xt_all[k_tok, r_tok, :, col_tok] = x[order].astype(mdt)
    xt = xt_all.reshape(NCORES, 128, TOTW)

    # wq[k, h, s, i, q, s', o] = w[e, i, o] on the s == s' diagonal
    e_half = e_band // 2
    e_sub = e_band % 2
    wqn = np.zeros((NCORES, 2, 2, F, NQUAD, 2, O), dtype=mdt)
    wqn[e_core, e_half, e_sub, :, e_quad, e_sub, :] = w.astype(mdt)
    wqk = wqn.reshape(NCORES, 128, NQUAD * 64)

    bpn = np.zeros((NCORES, 4, O, NQUAD), dtype=np.float32)
    bpn[e_core, e_band, :, e_quad] = b[:, 0, :]
    bpk = bpn.reshape(NCORES, 128, NQUAD)

    plan = (Q, X, TOTW)
    return plan, order, (k_tok, r_tok, col_tok), xt, wqk, bpk


def _unpack(results, tok_addr, order):
    k_tok, r_tok, col_tok = tok_addr
    ot = np.stack([results[k]["ot"] for k in range(NCORES)])  # [k, 128, TOTW]
    ot4 = ot.reshape(NCORES, 4, O, -1)  # [k, r, o, col]
    out = np.empty((N_TOK, O), dtype=np.float32)
    out[order] = ot4[k_tok, r_tok, :, col_tok]
    return out


def _prepare(x, inds, w, b):
    """Pack inputs and return (nc, in_maps, tok_addr, order)."""
    plan, order, tok_addr, xt, wqk, bpk = _pack(x, inds, w, b)
    Q, X, TOTW = plan
    key = (MM_DT, OT_DT, Q.tobytes())
    nc = _programs.get(key)
    if nc is None:
        nc = _build(Q, X, TOTW)
        _programs[key] = nc
    in_maps = [{"xt": xt[k], "wq": wqk[k], "bp": bpk[k]} for k in range(NCORES)]
    return nc, in_maps, tok_addr, order


def kernel(input, inds, w, b):
    x = np.ascontiguousarray(np.asarray(input, dtype=np.float32))
    inds = np.asarray(inds, dtype=np.int32)
    w = np.ascontiguousarray(np.asarray(w, dtype=np.float32))
    b = np.ascontiguousarray(np.asarray(b, dtype=np.float32))
    assert x.shape == (N_TOK, F) and inds.shape == (N_TOK,)
    assert w.shape == (E, F, O) and b.shape == (E, 1, O)

    try:
        nc, in_maps, tok_addr, order = _prepare(x, inds, w, b)
    except _CapacityOverflow:
        return (np.einsum("ni,nio->no", x, w[inds]) + b[inds, 0]).astype(np.float32)

    res = run_bass_kernel_spmd(nc, in_maps, list(range(NCORES)))
    return _unpack(res.results, tok_addr, order)


def last_program():
    """The most recently compiled Bass program (for profiling in test.py)."""
    return next(iter(_programs.values())) if _programs else None


# revision 15
# speedup vs baseline: 1.1062x; 1.1062x over previous
"""MoE routed expert matmul on 8 Trainium2 NeuronCores.

Problem: out[n] = input[n] @ w[inds[n]] + b[inds[n]]
  input [262144, 32] f32, inds [262144] i32 (1024 experts), w [1024, 32, 32], b [1024, 1, 32]

Strategy (K-stacked expert quads; host does routing/layout only — all FLOPs
on device):
  * Host sorts the 1024 experts by global token count (ascending) and chunks
    them into 32 quad-groups of 32 experts with near-equal counts.  Chunk q
    supplies one expert to each (core, band) pair: expert chunks[q][4k + r]
    goes to core k, quad q, band r (r in 0..3).  Every core runs the same
    program over its own 32 quads; quad q's column width Q[q] = max token
    count in the chunk (global max, so the SPMD shapes match), rounded up to
    8.  Count-matched chunks keep padding to a few percent.
  * Activation layout xt [128, TOTW] fp16: token t of (quad q, band r) sits
    at column X[q] + t, rows 32r..32r+32 (its 32 features).  Each column
    carries up to 4 tokens (one per band) — full 128-row density.
  * Weights upload as block-diagonal K=64 stacks (wq, 0.5 MB): for each quad
    and half h, a [64, 64] tile holds experts (q, 2h) and (q, 2h+1) on the
    diagonal.  Two [K=64, M=64, N=Q] matmuls per quad (tile_position (0,0) /
    (64,64)) then compute all 4 bands' tokens — each activation column
    streams through the PE twice instead of 4x (vs per-expert 32x32 tiles),
    and the off-diagonal zeros kill the cross-expert terms.
  * The PSUM result + per-quad bias column goes to an fp16 output tile
    (Scalar/Vector alternating), stored to DRAM in per-4-quad groups on
    alternating DMA rings (GpSimd SWDGE / Scalar HWDGE).  fp16 I/O halves
    DMA traffic vs f32; per-core HBM bytes ~4.8 MB -> ~13.3 us at 360 B/ns.
  * Host scatters the sorted outputs back to original token order.

Layouts (core k, quad q, band r = 2h + s, expert e = chunks[q][4k + r]):
  xt [128, TOTW]  xt[32r+i, X[q] + t]        = x[token t of e, feat i]  (fp16)
  wq [128, 2048]  wq[64h+32s+i, 64q+32s+o]   = w[e, i, o], 0 off-diag   (fp16)
  bp [128, 32]    bp[32r+o, q]               = b[e, 0, o]               (f32)
  ot [128, TOTW]  ot[32r+o, X[q] + t]        = out[token t of e, feat o](fp16)
"""

import numpy as np

import concourse.bass as bass
import concourse.mybir as mybir
import concourse.tile as tile
from concourse import bacc
from concourse.bass_utils import run_bass_kernel_spmd

N_TOK = 262144
E = 1024
F = 32
O = 32
NCORES = 8
NQUAD = 32  # quads per core; 4 experts each = 128 experts/core
GQ = 4  # quads per load/store group
NG = NQUAD // GQ
F32 = mybir.dt.float32
MM_DT = mybir.dt.float16
OT_DT = mybir.dt.float16

N_WARM = 8  # PE ramp warm-up matmuls
WARM_N = 160  # free-dim length of each warm-up matmul

_programs: dict[tuple, "bacc.Bacc"] = {}


class _CapacityOverflow(Exception):
    """A single expert got >512 tokens (~16 sigma out for uniform routing at
    256 tokens/expert).  Handled by a host fallback so kernel() still
    returns a correct result."""


def _plan(counts):
    """Chunk experts into count-matched quads; per-quad widths and offsets."""
    order_e = np.argsort(counts, kind="stable")  # ascending counts
    # chunk q holds 32 count-matched experts; descending so the pipeline
    # tail (last-stored groups) drains on the smallest transfers
    chunks = order_e.reshape(NQUAD, 32)[::-1]
    Q = np.maximum(16, ((counts[chunks[:, -1]] + 7) // 8) * 8)  # [NQUAD]
    # uniform width within each 4-quad group so one batched bias op (and one
    # contiguous PSUM slot stride) covers the group; count-sorted chunks make
    # the extra padding ~1%
    Q = np.repeat(Q.reshape(NG, GQ).max(axis=1), GQ)
    if Q.max() > 512:
        raise _CapacityOverflow(int(counts.max()))
    X = np.zeros(NQUAD + 1, dtype=np.int64)
    np.cumsum(Q, out=X[1:])
    TOTW = int(X[-1])
    j = np.arange(32)
    e_quad = np.empty(E, dtype=np.int64)
    e_core = np.empty(E, dtype=np.int64)
    e_band = np.empty(E, dtype=np.int64)
    e_quad[chunks] = np.arange(NQUAD)[:, None]
    e_core[chunks] = (j // 4)[None, :]
    e_band[chunks] = (j % 4)[None, :]
    return Q.astype(np.int64), X, TOTW, e_quad, e_core, e_band


def _build(Q, X, TOTW) -> "bacc.Bacc":
    nc = bacc.Bacc("TRN2", target_bir_lowering=False, debug=False, num_devices=NCORES)
    xt = nc.declare_dram_parameter("xt", [128, TOTW], MM_DT, isOutput=False)
    wq = nc.declare_dram_parameter("wq", [128, NQUAD * 64], MM_DT, isOutput=False)
    bp = nc.declare_dram_parameter("bp", [128, NQUAD], F32, isOutput=False)
    ot = nc.declare_dram_parameter("ot", [128, TOTW], OT_DT, isOutput=True)

    with tile.TileContext(nc) as tc:
        with (
            tc.tile_pool(name="const", bufs=1) as c_pool,
            tc.tile_pool(name="xt", bufs=4) as xt_pool,
            tc.tile_pool(name="out", bufs=4) as out_pool,
            tc.tile_pool(name="psm", bufs=2, space="PSUM") as psm_pool,
        ):
            wq_t = c_pool.tile([128, NQUAD * 64], MM_DT)
            bp_t = c_pool.tile([128, NQUAD], F32)
            warm_t = c_pool.tile([128, WARM_N], MM_DT)

            # loads: sync (SP HWDGE) carries the first wq half + all xt;
            # gpsimd (SWDGE) the second wq half and the bias
            wqh = NQUAD * 64 // 2
            nc.sync.dma_start(out=wq_t[:, :wqh], in_=wq[:, :wqh])
            nc.gpsimd.dma_start(out=wq_t[:, wqh:], in_=wq[:, wqh:])
            nc.gpsimd.dma_start(out=bp_t[:], in_=bp[:])

            # PE ramp warm-up on a memset scratch tile (PSUM never read)
            nc.vector.memset(warm_t[:], 0.0)
            warm_ps = psm_pool.tile(
                [128, WARM_N], F32, space="PSUM", name="warm_ps", tag="psm"
            )
            for _ in range(N_WARM):
                nc.tensor.matmul(
                    out=warm_ps[0:32, :],
                    lhsT=warm_t[0:32, 0:32],
                    rhs=warm_t[0:32, :],
                    start=True,
                    stop=True,
                    tile_position=(0, 0),
                )

            xt_tiles = {}
            o_tiles = {}

            def load_group(g, split=1):
                a, bnd = int(X[GQ * g]), int(X[GQ * (g + 1)])
                t = xt_pool.tile([128, bnd - a], MM_DT, name="xt_t", tag="xt_t")
                w_ = bnd - a
                for s in range(split):
                    c0, c1 = s * w_ // split, (s + 1) * w_ // split
                    nc.sync.dma_start(out=t[:, c0:c1], in_=xt[:, a + c0 : a + c1])
                xt_tiles[g] = t

            load_group(0, split=2)
            load_group(1)

            for g in range(NG):
                if g + 2 < NG:
                    load_group(g + 2)
                a, bnd = int(X[GQ * g]), int(X[GQ * (g + 1)])
                Qg = int(Q[GQ * g])
                o_t = out_pool.tile([128, bnd - a], OT_DT, name="o_t", tag="o_t")
                # one PSUM tile per group; each quad in its own 512-col bank
                psm = psm_pool.tile(
                    [128, GQ * 512], F32, space="PSUM", name="psm", tag="psm"
                )
                for qi in range(GQ):
                    q = GQ * g + qi
                    for h in range(2):
                        nc.tensor.matmul(
                            out=psm[64 * h : 64 * h + 64, 512 * qi : 512 * qi + Qg],
                            lhsT=wq_t[64 * h : 64 * h + 64, 64 * q : 64 * q + 64],
                            rhs=xt_tiles[g][
                                64 * h : 64 * h + 64, qi * Qg : (qi + 1) * Qg
                            ],
                            start=True,
                            stop=True,
                            tile_position=(64 * h, 64 * h),
                        )
                # batched bias + fp16 down-convert: ONE tensor_tensor per
                # group on DVE (broadcast bias view), ~0.8us vs 4 per-quad ops
                psm_view = psm[:, :].rearrange("p (c t) -> p c t", c=GQ)[:, :, :Qg]
                bias_view = bp_t[:, GQ * g : GQ * (g + 1), None].to_broadcast(
                    [128, GQ, Qg]
                )
                out_view = o_t[:, :].rearrange("p (c t) -> p c t", c=GQ)
                nc.vector.tensor_tensor(
                    out=out_view,
                    in0=psm_view,
                    in1=bias_view,
                    op=mybir.AluOpType.add,
                )
                ring = nc.gpsimd if g % 2 == 0 else nc.scalar
                ring.dma_start(out=ot[:, a:bnd], in_=o_t[:])

    nc.compile()
    return nc


def _pack(x, inds, w, b):
    """Host-side routing: sort tokens by expert, build per-core device arrays."""
    counts = np.bincount(inds, minlength=E)
    Q, X, TOTW, e_quad, e_core, e_band = _plan(counts)

    order = np.argsort(inds, kind="stable")
    sorted_inds = inds[order]
    starts = np.zeros(E, dtype=np.int64)
    np.cumsum(counts[:-1], out=starts[1:])
    slot = np.arange(N_TOK, dtype=np.int64) - starts[sorted_inds]

    k_tok = e_core[sorted_inds]
    r_tok = e_band[sorted_inds]
    col_tok = X[e_quad[sorted_inds]] + slot

    mdt = mybir.dt.np(MM_DT)
    xt_all = np.zeros((NCORES, 4, F, TOTW), dtype=mdt)
    xt_all[k_tok, r_tok, :, col_tok] = x[order].astype(mdt)
    xt = xt_all.reshape(NCORES, 128, TOTW)

    # wq[k, h, s, i, q, s', o] = w[e, i, o] on the s == s' diagonal
    e_half = e_band // 2
    e_sub = e_band % 2
    wqn = np.zeros((NCORES, 2, 2, F, NQUAD, 2, O), dtype=mdt)
    wqn[e_core, e_half, e_sub, :, e_quad, e_sub, :] = w.astype(mdt)
    wqk = wqn.reshape(NCORES, 128, NQUAD * 64)

    bpn = np.zeros((NCORES, 4, O, NQUAD), dtype=np.float32)
    bpn[e_core, e_band, :, e_quad] = b[:, 0, :]
    bpk = bpn.reshape(NCORES, 128, NQUAD)

    plan = (Q, X, TOTW)
    return plan, order, (k_tok, r_tok, col_tok), xt, wqk, bpk


def _unpack(results, tok_addr, order):
    k_tok, r_tok, col_tok = tok_addr
    ot = np.stack([results[k]["ot"] for k in range(NCORES)])  # [k, 128, TOTW]
    ot4 = ot.reshape(NCORES, 4, O, -1)  # [k, r, o, col]
    out = np.empty((N_TOK, O), dtype=np.float32)
    out[order] = ot4[k_tok, r_tok, :, col_tok]
    return out


def _prepare(x, inds, w, b):
    """Pack inputs and return (nc, in_maps, tok_addr, order)."""
    plan, order, tok_addr, xt, wqk, bpk = _pack(x, inds, w, b)
    Q, X, TOTW = plan
    key = (MM_DT, OT_DT, Q.tobytes())
    nc = _programs.get(key)
    if nc is None:
        nc = _build(Q, X, TOTW)
        _programs[key] = nc
    in_maps = [{"xt": xt[k], "wq": wqk[k], "bp": bpk[k]} for k in range(NCORES)]
    return nc, in_maps, tok_addr, order


def kernel(input, inds, w, b):
    x = np.ascontiguousarray(np.asarray(input, dtype=np.float32))
    inds = np.asarray(inds, dtype=np.int32)
    w = np.ascontiguousarray(np.asarray(w, dtype=np.float32))
    b = np.ascontiguousarray(np.asarray(b, dtype=np.float32))
    assert x.shape == (N_TOK, F) and inds.shape == (N_TOK,)
    assert w.shape == (E, F, O) and b.shape == (E, 1, O)

    try:
        nc, in_maps, tok_addr, order = _prepare(x, inds, w, b)
    except _CapacityOverflow:
        return (np.einsum("ni,nio->no", x, w[inds]) + b[inds, 0]).astype(np.float32)

    res = run_bass_kernel_spmd(nc, in_maps, list(range(NCORES)))
    return _unpack(res.results, tok_addr, order)


def last_program():
    """The most recently compiled Bass program (for profiling in test.py)."""
    return next(iter(_programs.values())) if _programs else None
